# revision 96
# baseline (speedup 1.0000x reference)
"""GQA (softermax) Trainium2 kernel, tensor-parallel over kv-head groups.

Problem: x[1,2048,4096], 32 q-heads / 8 kv-heads, head_dim=128, base-2
softmax (softermax), fp32 io. Each of the 8 cores owns one kv-head group
(4 q-heads, 512 q dims, 128 kv dims) and computes a full partial
o-projection [2048,4096] (fp16 staging); the host sums the 8 partials and
adds o_b.

All matmul operands are pre-transposed on the host and cast to bf16
(fp32 PSUM accumulation), so the device does no layout work except 16
V transposes. Per core:
  proj:  KT/VT/QT[d,s] = (wT chunk)^T @ xT chunk, bias via ACT; weights
         arrive host-packed in SBUF layout (>=1KB descriptor runs; sub-
         512B runs cost 2x DMA bandwidth) interleaved with x pieces in
         consumption order; the first seq block runs K/V/Q as 6
         concurrent PSUM accumulation streams so it consumes the mixed
         DMA stream at arrival rate; Q proj for heads 2-3 x seq-blocks
         2-3 is deferred into the first attention block's pipeline
         holes.
  attn:  S^T[k,q] = KT_chunk^T @ QT (bf16), with every S-pair emitted
         one kt-pair ahead of its exp so PV's wait on the exp is always
         covered by useful PE work; PT = exp2 via ACT over 1024-wide
         pairs -> bf16; O^T[d,q] = sum_kt Vn[kt]^T @ PT[kt];
         denominator: running DVE adds (f32r) + ones-matmul across
         partitions + DVE reciprocal + Pool partition_broadcast + DVE
         mul -> per-head OTb bf16; heads 0-2 defer this normalization
         past the next head's first S-pair, and the last head of each
         q block takes its last two kt-pairs straight from PT via bf16
         ones-matmuls so the chain that gates the next block's o-proj
         jobs doesn't wait on the final DVE adds.
  oproj: po[s,e] psum = sum_h OTb_h^T @ owT_h; one o-proj tile is
         emitted inside every attention kt-pair of the NEXT q block
         (software pipelining that fills the exp-latency hole between
         the S and PV matmuls); PSUM->SBUF copies alternate ACT/DVE into
         fp16 [128,1024] staging tiles, DMA'd 2 tiles per transfer.
"""

import math

import numpy as np
from ml_dtypes import bfloat16

import concourse.bass as bass
from concourse import bacc
import concourse.mybir as mybir
import concourse.tile as tile
from concourse.bass_utils import run_bass_kernel_spmd
from concourse.masks import make_identity

E = 4096          # embed dim
S = 2048          # sequence
D = 128           # head dim
NHL = 4           # q heads per core
DQ = NHL * D      # 512 q dims per core
DKV = 128         # kv dims per core (1 kv head)
NCORES = 8

SB = 512          # seq block for projection pass
NSB = S // SB
QS = 512          # q block in attention
NQS = S // QS
NKT = S // 128    # 16 k chunks
NKP = NKT // 2    # 8 k chunk pairs
NE = E // 128     # 32 embed chunks
EC = 512          # oproj out-column block
NEC = E // EC

F32 = mybir.dt.float32
F16 = mybir.dt.float16
BF16 = mybir.dt.bfloat16
F32R = mybir.dt.float32r
EXP_SCALE = math.log(2.0) / math.sqrt(D)

_CACHED_NC = None


def r(ap):
    return ap.bitcast(F32R)


def build_bass():
    nc = bacc.Bacc(None)

    xT_d = nc.declare_dram_parameter("xT", [E, S], BF16, isOutput=False)
    # weights arrive pre-packed in SBUF layout so every DMA descriptor is a
    # >=1KB contiguous run (sub-512B runs transfer at half bandwidth)
    qwp01_d = nc.declare_dram_parameter("qwp01", [128, NE, 2 * D], BF16,
                                        isOutput=False)
    qwp23_d = nc.declare_dram_parameter("qwp23", [128, NE, 2 * D], BF16,
                                        isOutput=False)
    kwp_d = nc.declare_dram_parameter("kwp", [128, NE, DKV], BF16,
                                      isOutput=False)
    vwp_d = nc.declare_dram_parameter("vwp", [128, NE, DKV], BF16,
                                      isOutput=False)
    owT_d = nc.declare_dram_parameter("owT", [DQ, E], BF16, isOutput=False)
    qb_d = nc.declare_dram_parameter("qb", [DQ], F32, isOutput=False)
    kb_d = nc.declare_dram_parameter("kb", [DKV], F32, isOutput=False)
    vb_d = nc.declare_dram_parameter("vb", [DKV], F32, isOutput=False)
    out_d = nc.declare_dram_parameter("out", [S, E], F16, isOutput=True)

    Id = mybir.ActivationFunctionType.Identity
    Exp = mybir.ActivationFunctionType.Exp

    with tile.TileContext(nc) as tc:
        with (
            tc.tile_pool(name="consts", bufs=1) as consts,
            tc.tile_pool(name="weights", bufs=1) as wpool,
            tc.tile_pool(name="persist", bufs=1) as persist,
        ):
            # ---------------- constants ----------------
            identw = consts.tile([128, 128], BF16)
            make_identity(nc, identw[:, :])
            ones_f = consts.tile([128, 1], F32)
            nc.gpsimd.memset(ones_f[:, :], 1.0)
            ones_col = consts.tile([128, 1], F32R)
            nc.vector.tensor_copy(ones_col[:, :], ones_f[:, :])
            ones_b = consts.tile([128, 1], BF16)
            nc.gpsimd.memset(ones_b[:, :], 1.0)

            qb_sb = consts.tile([128, NHL], F32)
            kb_sb = consts.tile([128, 1], F32)
            vb_sb = consts.tile([128, 1], F32)

            # ---------------- persistent activations ----------------
            KT = persist.tile([128, S], BF16)            # K^T [d, seq]
            Vn = persist.tile([128, NKT, 128], BF16)     # V natural [seq, d]
            QT = persist.tile([128, NHL, S], BF16)       # Q^T per head [d, seq]

            # weight tiles (DMAs interleaved with x blocks, see below)
            kwT = wpool.tile([128, NE, DKV], BF16)
            vwT = wpool.tile([128, NE, DKV], BF16)
            qwT23 = wpool.tile([128, NE, 2 * D], BF16)
            owT = wpool.tile([128, NHL, E], BF16)

            # ---------------- projections ----------------
            with tc.tile_pool(name="xz", bufs=2) as xz:
              with (
                tc.tile_pool(name="wq012", bufs=1) as wq012,
                tc.tile_pool(name="ps_acc", bufs=6, space="PSUM") as ps_acc,
                tc.tile_pool(name="ps_tr", bufs=2, space="PSUM") as ps_tr,
              ):
                qwT = wq012.tile([128, NE, 2 * D], BF16)
                VT = wq012.tile([128, S], BF16)
                # prefetch order: kw/x0 interleaved by quarter, vw, qw per
                # head, x1..x3, owT last — the first K matmul waits on ~0.5 MB
                # and nothing bulky delays the x blocks it gates.
                xts = {}

                def xt_dma(sb, interleave=(), pieces=4):
                    xt = xz.tile([128, NE, SB], BF16, tag="xt", name=f"xt{sb}")
                    ssl = slice(sb * SB, (sb + 1) * SB)
                    ne_p = NE // pieces

                    def wq(dst, wsrc, q):
                        nc.sync.dma_start(
                            dst[:, q * ne_p:(q + 1) * ne_p, :],
                            wsrc[:, q * ne_p:(q + 1) * ne_p, :])

                    for q in range(pieces):
                        # first weight, then x (so the first matmuls can
                        # start), then the remaining weights of this piece
                        if interleave:
                            wq(*interleave[0], q)
                        nc.sync.dma_start(
                            xt[:, q * ne_p:(q + 1) * ne_p, :],
                            xT_d[q * ne_p * 128:(q + 1) * ne_p * 128,
                                 ssl].rearrange("(ne p) s -> p ne s", p=128))
                        for dst, wsrc in interleave[1:]:
                            wq(dst, wsrc, q)
                    xts[sb] = xt

                wq_list = [(kwT, kwp_d), (qwT, qwp01_d),
                           (qwT23, qwp23_d), (vwT, vwp_d)]
                xt_dma(0, interleave=wq_list, pieces=8)
                nc.sync.dma_start(kb_sb[:, :],
                                  kb_d[:].rearrange("(p o) -> p o", o=1))
                nc.sync.dma_start(vb_sb[:, :],
                                  vb_d[:].rearrange("(p o) -> p o", o=1))
                nc.sync.dma_start(qb_sb[:, :],
                                  qb_d[:].rearrange("(t p) -> p t", p=128))
                xt_dma(1)

                for sb in range(NSB):
                    if sb >= 2:
                        xt_dma(sb)
                    if sb == NSB - 1:
                        nc.sync.dma_start(
                            owT[:, :, :],
                            owT_d[:, :].rearrange("(h p) e -> p h e", p=128))
                    xt = xts[sb]
                    ssl = slice(sb * SB, (sb + 1) * SB)
                    qsrcs = [(qwT[:, :, h * 128:(h + 1) * 128] if h < 2
                              else qwT23[:, :, (h - 2) * 128:(h - 1) * 128])
                             for h in range(NHL)]

                    if sb == 0:
                        # 6 concurrent accumulation streams so sb0 consumes
                        # each DMA'd e-chunk eighth for K, V and all Q heads
                        # at once (pure DMA-rate-bound, no ordering stalls)
                        streams = [kwT] + qsrcs + [vwT]
                        pss = [ps_acc.tile([128, SB], F32, tag="acc",
                                           name=f"ps6_{j}")
                               for j in range(6)]
                        for q8 in range(8):
                            for j, w in enumerate(streams):
                                for e in range(q8 * 4, q8 * 4 + 4):
                                    nc.tensor.matmul(
                                        pss[j][:, :], w[:, e, :], xt[:, e, :],
                                        start=(e == 0), stop=(e == NE - 1))
                        nc.scalar.activation(KT[:, ssl], pss[0][:, :], Id,
                                             bias=kb_sb[:, 0:1])
                        for h in range(NHL):
                            nc.scalar.activation(QT[:, h, ssl],
                                                 pss[1 + h][:, :], Id,
                                                 bias=qb_sb[:, h:h + 1])
                        nc.scalar.activation(VT[:, ssl], pss[5][:, :], Id,
                                             bias=vb_sb[:, 0:1])
                    else:
                        ps_k = ps_acc.tile([128, SB], F32, tag="acc")
                        for e in range(NE):
                            nc.tensor.matmul(ps_k[:, :], kwT[:, e, :],
                                             xt[:, e, :],
                                             start=(e == 0), stop=(e == NE - 1))
                        nc.scalar.activation(KT[:, ssl], ps_k[:, :], Id,
                                             bias=kb_sb[:, 0:1])

                        ps_v = ps_acc.tile([128, SB], F32, tag="acc")
                        for e in range(NE):
                            nc.tensor.matmul(ps_v[:, :], vwT[:, e, :],
                                             xt[:, e, :],
                                             start=(e == 0), stop=(e == NE - 1))
                        nc.scalar.activation(VT[:, ssl], ps_v[:, :], Id,
                                             bias=vb_sb[:, 0:1])

                        for h in range(NHL):
                            if h >= 2 and sb >= 2:
                                continue   # deferred into qi0's slots
                            ps_q = ps_acc.tile([128, SB], F32, tag="acc")
                            for e in range(NE):
                                nc.tensor.matmul(
                                    ps_q[:, :], qsrcs[h][:, e, :], xt[:, e, :],
                                    start=(e == 0), stop=(e == NE - 1))
                            nc.scalar.activation(QT[:, h, ssl], ps_q[:, :], Id,
                                                 bias=qb_sb[:, h:h + 1])

                    for i in range(SB // 128):
                        t = sb * (SB // 128) + i
                        tp = ps_tr.tile([128, 128], BF16, tag="tr")
                        nc.tensor.transpose(
                            tp[:, :], VT[:, t * 128:(t + 1) * 128], identw[:, :])
                        nc.vector.tensor_copy(Vn[:, t, :], tp[:, :])

              # ---------------- attention + pipelined o-projection --------
              with (
                  tc.tile_pool(name="attn", bufs=2) as attn,
                  tc.tile_pool(name="attn1", bufs=1) as attn1,
                  tc.tile_pool(name="obp", bufs=4) as obp,
                  tc.tile_pool(name="ps_s", bufs=2, space="PSUM") as ps_s,
                  tc.tile_pool(name="ps_o", bufs=2, space="PSUM") as ps_o,
                  tc.tile_pool(name="ps_po", bufs=2, space="PSUM") as ps_po,
              ):
                ncopy = [0]
                ob_cur = [None]

                def oproj_tile(qi, OTb_q, sl, ec, split_dma=False):
                    # 2 po tiles land in one [128, 2*EC] SBUF tile -> 1 DMA
                    st0 = qi * QS + sl * 128
                    po = ps_po.tile([128, EC], F32, tag="po")
                    for dh in range(NHL):
                        nc.tensor.matmul(
                            po[:, :],
                            OTb_q[dh][:, sl * 128:(sl + 1) * 128],
                            owT[:, dh, ec * EC:(ec + 1) * EC],
                            start=(dh == 0), stop=(dh == NHL - 1))
                    if ec % 2 == 0:
                        ob_cur[0] = obp.tile([128, 2 * EC], F16, tag="ob",
                                             name="ob")
                    ob = ob_cur[0]
                    osl = slice((ec % 2) * EC, (ec % 2 + 1) * EC)
                    i = ncopy[0]
                    ncopy[0] += 1
                    if i % 2 == 0:
                        nc.scalar.copy(ob[:, osl], po[:, :])
                    else:
                        nc.vector.tensor_copy(ob[:, osl], po[:, :])
                    if split_dma:
                        nc.sync.dma_start(
                            out_d[st0:st0 + 128, ec * EC:(ec + 1) * EC],
                            ob[:, osl])
                    elif ec % 2 == 1:
                        nc.sync.dma_start(
                            out_d[st0:st0 + 128,
                                  (ec - 1) * EC:(ec + 1) * EC], ob[:, :])

                def deferred_q_jobs(sb, h):
                    # Q proj deferred from the projection phase, run inside
                    # qi0's fill slots; borrows the idle po PSUM ring and
                    # the still-resident xt tiles.
                    xt = xts[sb]
                    ssl = slice(sb * SB, (sb + 1) * SB)
                    qsrc = qwT23[:, :, (h - 2) * 128:(h - 1) * 128]
                    st = {}

                    def chunk(ci):
                        if ci == 0:
                            st["ps"] = ps_po.tile([128, SB], F32, tag="po",
                                                  name="ps_qd")
                        ps_q = st["ps"]
                        for e in range(ci * 4, ci * 4 + 4):
                            nc.tensor.matmul(
                                ps_q[:, :], qsrc[:, e, :], xt[:, e, :],
                                start=(e == 0), stop=(e == NE - 1))
                        if ci == NE // 4 - 1:
                            nc.scalar.activation(
                                QT[:, h, ssl], ps_q[:, :], Id,
                                bias=qb_sb[:, h:h + 1])

                    return [(lambda ci=ci: chunk(ci)) for ci in range(NE // 4)]

                OTb_prev = None
                pending = [None]

                units = [(uq, uh, ukp) for uq in range(NQS)
                         for uh in range(NHL) for ukp in range(NKP)]

                def emit_S(u):
                    uq, uh, ukp = u
                    sps = ps_s.tile([128, 2 * QS], F32, tag="s", name="sps")
                    uqsl = slice(uq * QS, (uq + 1) * QS)
                    for j in (0, 1):
                        kk = 2 * ukp + j
                        nc.tensor.matmul(sps[:, j * QS:(j + 1) * QS],
                                         KT[:, kk * 128:(kk + 1) * 128],
                                         QT[:, uh, uqsl],
                                         start=True, stop=True)
                    return sps

                sps_next = [None]
                unit_idx = [0]
                OTb = None

                def finalize():
                    pqi, ph, ops, acc, pPT = pending[0]
                    pending[0] = None
                    tgt = OTb if pqi == qi_cur[0] else OTb_prev
                    sums = ps_s.tile([1, QS], F32, tag="s", name="sums")
                    last = (ph == NHL - 1)
                    if last:
                        # tail-exposed: last 2 pairs' contribution straight
                        # from PT so sums doesn't wait on their DVE adds
                        nc.tensor.matmul(sums[:, :], ones_col[:, :],
                                         acc[:, :], start=True, stop=False)
                        for kk in range(NKT - 4, NKT):
                            nc.tensor.matmul(sums[:, :], ones_b[:, :],
                                             pPT[:, kk, :], start=False,
                                             stop=(kk == NKT - 1))
                    else:
                        nc.tensor.matmul(sums[:, :], ones_col[:, :],
                                         acc[:, :], start=True, stop=True)
                    recip = attn1.tile([1, QS], F32, tag="recip")
                    bc = attn1.tile([128, QS], F32, tag="bc")
                    tgt[ph] = attn.tile([128, QS], BF16, tag=f"OTb{ph}",
                                        name=f"OTb{ph}")
                    # pipeline the normalization chain in column quarters
                    # across DVE/Pool (the last head's chain gates the next
                    # block's o-proj jobs)
                    for ci in range(4):
                        c = slice(ci * QS // 4, (ci + 1) * QS // 4)
                        nc.vector.reciprocal(recip[:, c], sums[:, c])
                        nc.gpsimd.partition_broadcast(bc[:, c], recip[:, c])
                        nc.vector.tensor_mul(tgt[ph][:, c], ops[:, c],
                                             bc[:, c])

                qi_cur = [0]
                for qi in range(NQS):
                    qi_cur[0] = qi
                    qsl = slice(qi * QS, (qi + 1) * QS)
                    if qi == 0:
                        jobs = (deferred_q_jobs(2, 2)
                                + deferred_q_jobs(2, 3)
                                + deferred_q_jobs(3, 2)
                                + deferred_q_jobs(3, 3))
                    else:
                        # o-projection of the previous q block, interleaved
                        jobs = [
                            (lambda sl=sl, ec=ec, prev=OTb_prev, pqi=qi - 1:
                             oproj_tile(pqi, prev, sl, ec))
                            for sl in range(QS // 128) for ec in range(NEC)]
                    ji = 0
                    OTb = [None] * NHL
                    for h in range(NHL):
                        PT = attn.tile([128, NKT, QS], BF16, tag="PT")
                        ops = ps_o.tile([128, QS], F32, tag="o")
                        acc = attn1.tile([128, QS], F32R, tag="pacc")
                        for kp in range(NKP):
                            k0, k1 = 2 * kp, 2 * kp + 1
                            # S-pairs are emitted one unit ahead (below), so
                            # the exp for this pair was already fed
                            ui = unit_idx[0]
                            unit_idx[0] += 1
                            if ui == 0:
                                sps_next[0] = emit_S(units[0])
                            sps = sps_next[0]
                            nc.scalar.activation(
                                PT[:, k0:k0 + 2, :].rearrange("p k q -> p (k q)"),
                                sps[:, :], Exp, scale=EXP_SCALE)
                            if ui + 1 < len(units):
                                sps_next[0] = emit_S(units[ui + 1])
                            if kp == 0 and pending[0] is not None:
                                # normalization of the previous head, deferred
                                # here so its DVE wait hides under this exp
                                finalize()
                            # fill the exp-latency hole between S and PV
                            if ji < len(jobs):
                                jobs[ji]()
                                ji += 1
                            nc.tensor.matmul(ops[:, :], Vn[:, k0, :],
                                             PT[:, k0, :],
                                             start=(kp == 0), stop=False)
                            nc.tensor.matmul(ops[:, :], Vn[:, k1, :],
                                             PT[:, k1, :],
                                             start=False, stop=(kp == NKP - 1))
                            skip_last = (h == NHL - 1 and kp >= NKP - 2)
                            if kp == 0:
                                nc.vector.tensor_add(acc[:, :], PT[:, 0, :],
                                                     PT[:, 1, :])
                            elif not skip_last:
                                nc.vector.tensor_add(acc[:, :],
                                                     acc[:, :].bitcast(F32),
                                                     PT[:, k0, :])
                                nc.vector.tensor_add(acc[:, :],
                                                     acc[:, :].bitcast(F32),
                                                     PT[:, k1, :])
                        pending[0] = (qi, h, ops, acc, PT)
                        if h == NHL - 1:
                            # finalize immediately: the next q block's first
                            # o-proj jobs need this head's OTb
                            finalize()
                    OTb_prev = OTb

                # last q block's o-projection; final pair DMAs per-tile so
                # the drain overlaps the last copies
                for sl in range(QS // 128):
                    for ec in range(NEC):
                        oproj_tile(NQS - 1, OTb_prev, sl, ec,
                                   split_dma=(sl == QS // 128 - 1 and ec >= 6))

    nc.finalize()
    return nc


def make_in_maps(x, q_w, q_b, k_w, k_b, v_w, v_b, o_w):
    x2 = np.asarray(x, np.float32).reshape(S, E)
    xT = np.ascontiguousarray(x2.T).astype(bfloat16)
    q_w = np.asarray(q_w, np.float32)
    k_w = np.asarray(k_w, np.float32)
    v_w = np.asarray(v_w, np.float32)
    o_w = np.asarray(o_w, np.float32)
    in_maps = []
    for c in range(NCORES):
        qsl = slice(c * DQ, (c + 1) * DQ)
        ksl = slice(c * DKV, (c + 1) * DKV)
        qwT = q_w[qsl].T.astype(bfloat16)                  # [E, 512]

        def pack(w):
            # [E, d] -> SBUF layout [128, NE, d]: (p, ne, j) = w[ne*128+p, j]
            d = w.shape[1]
            return np.ascontiguousarray(
                w.reshape(NE, 128, d).transpose(1, 0, 2))

        in_maps.append({
            "xT": xT,
            "qwp01": pack(qwT[:, 0:2 * D]),
            "qwp23": pack(qwT[:, 2 * D:4 * D]),
            "qb": np.ascontiguousarray(np.asarray(q_b, np.float32)[qsl]),
            "kwp": pack(k_w[ksl].T.astype(bfloat16)),
            "kb": np.ascontiguousarray(np.asarray(k_b, np.float32)[ksl]),
            "vwp": pack(v_w[ksl].T.astype(bfloat16)),
            "vb": np.ascontiguousarray(np.asarray(v_b, np.float32)[ksl]),
            "owT": np.ascontiguousarray(o_w[:, qsl].T).astype(bfloat16),
        })
    return in_maps


def kernel(x, q_w, q_b, k_w, k_b, v_w, v_b, o_w, o_b):
    global _CACHED_NC
    in_maps = make_in_maps(x, q_w, q_b, k_w, k_b, v_w, v_b, o_w)
    if _CACHED_NC is None:
        _CACHED_NC = build_bass()
    res = run_bass_kernel_spmd(_CACHED_NC, in_maps, list(range(NCORES)))
    out = np.zeros((S, E), np.float64)
    for i in range(NCORES):
        out += res.results[i]["out"].astype(np.float64)
    out += np.asarray(o_b, np.float64)
    return out.astype(np.float32).reshape(1, S, E)


# revision 97
# speedup vs baseline: 1.0074x; 1.0074x over previous
"""GQA (softermax) Trainium2 kernel, tensor-parallel over kv-head groups.

Problem: x[1,2048,4096], 32 q-heads / 8 kv-heads, head_dim=128, base-2
softmax (softermax), fp32 io. Each of the 8 cores owns one kv-head group
(4 q-heads, 512 q dims, 128 kv dims) and computes a full partial
o-projection [2048,4096] (fp16 staging); the host sums the 8 partials and
adds o_b.

All matmul operands are pre-transposed on the host and cast to bf16
(fp32 PSUM accumulation), so the device does no layout work except 16
V transposes. Per core:
  proj:  KT/VT/QT[d,s] = (wT chunk)^T @ xT chunk, bias via ACT; weights
         arrive host-packed in SBUF layout (>=1KB descriptor runs; sub-
         512B runs cost 2x DMA bandwidth) interleaved with x pieces in
         consumption order; the first seq block runs K/V/Q as 6
         concurrent PSUM accumulation streams so it consumes the mixed
         DMA stream at arrival rate; Q proj for heads 2-3 x seq-blocks
         2-3 is deferred into the first attention block's pipeline
         holes.
  attn:  S^T[k,q] = KT_chunk^T @ QT (bf16), with every S-pair emitted
         one kt-pair ahead of its exp so PV's wait on the exp is always
         covered by useful PE work; PT = exp2 via ACT over 1024-wide
         pairs -> bf16; O^T[d,q] = sum_kt Vn[kt]^T @ PT[kt];
         denominator: running DVE adds (f32r) + ones-matmul across
         partitions + DVE reciprocal + Pool partition_broadcast + DVE
         mul -> per-head OTb bf16; heads 0-2 defer this normalization
         past the next head's first S-pair, and the last head of each
         q block takes its last two kt-pairs straight from PT via bf16
         ones-matmuls so the chain that gates the next block's o-proj
         jobs doesn't wait on the final DVE adds.
  oproj: po[s,e] psum = sum_h OTb_h^T @ owT_h; one o-proj tile is
         emitted inside every attention kt-pair of the NEXT q block
         (software pipelining that fills the exp-latency hole between
         the S and PV matmuls); PSUM->SBUF copies alternate ACT/DVE into
         fp16 [128,1024] staging tiles, DMA'd 2 tiles per transfer.
"""

import math

import numpy as np
from ml_dtypes import bfloat16

import concourse.bass as bass
from concourse import bacc
import concourse.mybir as mybir
import concourse.tile as tile
from concourse.bass_utils import run_bass_kernel_spmd
from concourse.masks import make_identity

E = 4096          # embed dim
S = 2048          # sequence
D = 128           # head dim
NHL = 4           # q heads per core
DQ = NHL * D      # 512 q dims per core
DKV = 128         # kv dims per core (1 kv head)
NCORES = 8

SB = 512          # seq block for projection pass
NSB = S // SB
QS = 512          # q block in attention
NQS = S // QS
NKT = S // 128    # 16 k chunks
NKP = NKT // 2    # 8 k chunk pairs
NE = E // 128     # 32 embed chunks
EC = 512          # oproj out-column block
NEC = E // EC

F32 = mybir.dt.float32
F16 = mybir.dt.float16
BF16 = mybir.dt.bfloat16
F32R = mybir.dt.float32r
EXP_SCALE = math.log(2.0) / math.sqrt(D)

_CACHED_NC = None


def r(ap):
    return ap.bitcast(F32R)


def build_bass():
    nc = bacc.Bacc(None)

    xT_d = nc.declare_dram_parameter("xT", [E, S], BF16, isOutput=False)
    # weights arrive pre-packed in SBUF layout so every DMA descriptor is a
    # >=1KB contiguous run (sub-512B runs transfer at half bandwidth)
    qwp01_d = nc.declare_dram_parameter("qwp01", [128, NE, 2 * D], BF16,
                                        isOutput=False)
    qwp23_d = nc.declare_dram_parameter("qwp23", [128, NE, 2 * D], BF16,
                                        isOutput=False)
    kwp_d = nc.declare_dram_parameter("kwp", [128, NE, DKV], BF16,
                                      isOutput=False)
    vwp_d = nc.declare_dram_parameter("vwp", [128, NE, DKV], BF16,
                                      isOutput=False)
    owT_d = nc.declare_dram_parameter("owT", [DQ, E], BF16, isOutput=False)
    qb_d = nc.declare_dram_parameter("qb", [DQ], F32, isOutput=False)
    kb_d = nc.declare_dram_parameter("kb", [DKV], F32, isOutput=False)
    vb_d = nc.declare_dram_parameter("vb", [DKV], F32, isOutput=False)
    out_d = nc.declare_dram_parameter("out", [S, E], F16, isOutput=True)

    Id = mybir.ActivationFunctionType.Identity
    Exp = mybir.ActivationFunctionType.Exp

    with tile.TileContext(nc) as tc:
        with (
            tc.tile_pool(name="consts", bufs=1) as consts,
            tc.tile_pool(name="weights", bufs=1) as wpool,
            tc.tile_pool(name="persist", bufs=1) as persist,
        ):
            # ---------------- constants ----------------
            identw = consts.tile([128, 128], BF16)
            make_identity(nc, identw[:, :])
            ones_f = consts.tile([128, 1], F32)
            nc.gpsimd.memset(ones_f[:, :], 1.0)
            ones_col = consts.tile([128, 1], F32R)
            nc.vector.tensor_copy(ones_col[:, :], ones_f[:, :])
            ones_b = consts.tile([128, 1], BF16)
            nc.gpsimd.memset(ones_b[:, :], 1.0)

            qb_sb = consts.tile([128, NHL], F32)
            kb_sb = consts.tile([128, 1], F32)
            vb_sb = consts.tile([128, 1], F32)

            # ---------------- persistent activations ----------------
            KT = persist.tile([128, S], BF16)            # K^T [d, seq]
            Vn = persist.tile([128, NKT, 128], BF16)     # V natural [seq, d]
            QT = persist.tile([128, NHL, S], BF16)       # Q^T per head [d, seq]

            # weight tiles (DMAs interleaved with x blocks, see below)
            kwT = wpool.tile([128, NE, DKV], BF16)
            vwT = wpool.tile([128, NE, DKV], BF16)
            qwT23 = wpool.tile([128, NE, 2 * D], BF16)
            owT = wpool.tile([128, NHL, E], BF16)

            # ---------------- projections ----------------
            with tc.tile_pool(name="xz", bufs=2) as xz:
              with (
                tc.tile_pool(name="wq012", bufs=1) as wq012,
                tc.tile_pool(name="ps_acc", bufs=6, space="PSUM") as ps_acc,
                tc.tile_pool(name="ps_tr", bufs=2, space="PSUM") as ps_tr,
              ):
                qwT = wq012.tile([128, NE, 2 * D], BF16)
                VT = wq012.tile([128, S], BF16)
                # prefetch order: kw/x0 interleaved by quarter, vw, qw per
                # head, x1..x3, owT last — the first K matmul waits on ~0.5 MB
                # and nothing bulky delays the x blocks it gates.
                xts = {}

                def xt_dma(sb, interleave=(), pieces=4):
                    xt = xz.tile([128, NE, SB], BF16, tag="xt", name=f"xt{sb}")
                    ssl = slice(sb * SB, (sb + 1) * SB)
                    ne_p = NE // pieces

                    def wq(dst, wsrc, q):
                        nc.sync.dma_start(
                            dst[:, q * ne_p:(q + 1) * ne_p, :],
                            wsrc[:, q * ne_p:(q + 1) * ne_p, :])

                    for q in range(pieces):
                        # first weight, then x (so the first matmuls can
                        # start), then the remaining weights of this piece
                        if interleave:
                            wq(*interleave[0], q)
                        nc.sync.dma_start(
                            xt[:, q * ne_p:(q + 1) * ne_p, :],
                            xT_d[q * ne_p * 128:(q + 1) * ne_p * 128,
                                 ssl].rearrange("(ne p) s -> p ne s", p=128))
                        for dst, wsrc in interleave[1:]:
                            wq(dst, wsrc, q)
                    xts[sb] = xt

                wq_list = [(kwT, kwp_d), (qwT, qwp01_d),
                           (qwT23, qwp23_d), (vwT, vwp_d)]
                xt_dma(0, interleave=wq_list, pieces=8)
                nc.sync.dma_start(kb_sb[:, :],
                                  kb_d[:].rearrange("(p o) -> p o", o=1))
                nc.sync.dma_start(vb_sb[:, :],
                                  vb_d[:].rearrange("(p o) -> p o", o=1))
                nc.sync.dma_start(qb_sb[:, :],
                                  qb_d[:].rearrange("(t p) -> p t", p=128))
                xt_dma(1)

                for sb in range(NSB):
                    if sb >= 2:
                        xt_dma(sb)
                    if sb == NSB - 1:
                        nc.sync.dma_start(
                            owT[:, :, :],
                            owT_d[:, :].rearrange("(h p) e -> p h e", p=128))
                    xt = xts[sb]
                    ssl = slice(sb * SB, (sb + 1) * SB)
                    qsrcs = [(qwT[:, :, h * 128:(h + 1) * 128] if h < 2
                              else qwT23[:, :, (h - 2) * 128:(h - 1) * 128])
                             for h in range(NHL)]

                    if sb == 0:
                        # 6 concurrent accumulation streams so sb0 consumes
                        # each DMA'd e-chunk eighth for K, V and all Q heads
                        # at once (pure DMA-rate-bound, no ordering stalls)
                        streams = [kwT] + qsrcs + [vwT]
                        pss = [ps_acc.tile([128, SB], F32, tag="acc",
                                           name=f"ps6_{j}")
                               for j in range(6)]
                        for q8 in range(8):
                            for j, w in enumerate(streams):
                                for e in range(q8 * 4, q8 * 4 + 4):
                                    nc.tensor.matmul(
                                        pss[j][:, :], w[:, e, :], xt[:, e, :],
                                        start=(e == 0), stop=(e == NE - 1))
                        nc.scalar.activation(KT[:, ssl], pss[0][:, :], Id,
                                             bias=kb_sb[:, 0:1])
                        for h in range(NHL):
                            nc.scalar.activation(QT[:, h, ssl],
                                                 pss[1 + h][:, :], Id,
                                                 bias=qb_sb[:, h:h + 1])
                        nc.scalar.activation(VT[:, ssl], pss[5][:, :], Id,
                                             bias=vb_sb[:, 0:1])
                    else:
                        ps_k = ps_acc.tile([128, SB], F32, tag="acc")
                        for e in range(NE):
                            nc.tensor.matmul(ps_k[:, :], kwT[:, e, :],
                                             xt[:, e, :],
                                             start=(e == 0), stop=(e == NE - 1))
                        nc.scalar.activation(KT[:, ssl], ps_k[:, :], Id,
                                             bias=kb_sb[:, 0:1])

                        ps_v = ps_acc.tile([128, SB], F32, tag="acc")
                        for e in range(NE):
                            nc.tensor.matmul(ps_v[:, :], vwT[:, e, :],
                                             xt[:, e, :],
                                             start=(e == 0), stop=(e == NE - 1))
                        nc.scalar.activation(VT[:, ssl], ps_v[:, :], Id,
                                             bias=vb_sb[:, 0:1])

                        for h in range(NHL):
                            if h >= 2 and sb >= 2:
                                continue   # deferred into qi0's slots
                            ps_q = ps_acc.tile([128, SB], F32, tag="acc")
                            for e in range(NE):
                                nc.tensor.matmul(
                                    ps_q[:, :], qsrcs[h][:, e, :], xt[:, e, :],
                                    start=(e == 0), stop=(e == NE - 1))
                            nc.scalar.activation(QT[:, h, ssl], ps_q[:, :], Id,
                                                 bias=qb_sb[:, h:h + 1])

                    for i in range(SB // 128):
                        t = sb * (SB // 128) + i
                        tp = ps_tr.tile([128, 128], BF16, tag="tr")
                        nc.tensor.transpose(
                            tp[:, :], VT[:, t * 128:(t + 1) * 128], identw[:, :])
                        nc.vector.tensor_copy(Vn[:, t, :], tp[:, :])

              # ---------------- attention + pipelined o-projection --------
              with (
                  tc.tile_pool(name="attn", bufs=2) as attn,
                  tc.tile_pool(name="attn1", bufs=1) as attn1,
                  tc.tile_pool(name="obp", bufs=4) as obp,
                  tc.tile_pool(name="ps_s", bufs=2, space="PSUM") as ps_s,
                  tc.tile_pool(name="ps_o", bufs=2, space="PSUM") as ps_o,
                  tc.tile_pool(name="ps_po", bufs=2, space="PSUM") as ps_po,
              ):
                ncopy = [0]
                ob_cur = [None]

                def oproj_tile(qi, OTb_q, sl, ec, split_dma=False):
                    # 2 po tiles land in one [128, 2*EC] SBUF tile -> 1 DMA
                    st0 = qi * QS + sl * 128
                    po = ps_po.tile([128, EC], F32, tag="po")
                    for dh in range(NHL):
                        nc.tensor.matmul(
                            po[:, :],
                            OTb_q[dh][:, sl * 128:(sl + 1) * 128],
                            owT[:, dh, ec * EC:(ec + 1) * EC],
                            start=(dh == 0), stop=(dh == NHL - 1))
                    if ec % 2 == 0:
                        ob_cur[0] = obp.tile([128, 2 * EC], F16, tag="ob",
                                             name="ob")
                    ob = ob_cur[0]
                    osl = slice((ec % 2) * EC, (ec % 2 + 1) * EC)
                    i = ncopy[0]
                    ncopy[0] += 1
                    # all copies on ACT: on DVE they would queue ahead of the
                    # adds chain that anchors each head's normalization
                    nc.scalar.copy(ob[:, osl], po[:, :])
                    if split_dma:
                        nc.sync.dma_start(
                            out_d[st0:st0 + 128, ec * EC:(ec + 1) * EC],
                            ob[:, osl])
                    elif ec % 2 == 1:
                        nc.sync.dma_start(
                            out_d[st0:st0 + 128,
                                  (ec - 1) * EC:(ec + 1) * EC], ob[:, :])

                def deferred_q_jobs(sb, h):
                    # Q proj deferred from the projection phase, run inside
                    # qi0's fill slots; borrows the idle po PSUM ring and
                    # the still-resident xt tiles.
                    xt = xts[sb]
                    ssl = slice(sb * SB, (sb + 1) * SB)
                    qsrc = qwT23[:, :, (h - 2) * 128:(h - 1) * 128]
                    st = {}

                    def chunk(ci):
                        if ci == 0:
                            st["ps"] = ps_po.tile([128, SB], F32, tag="po",
                                                  name="ps_qd")
                        ps_q = st["ps"]
                        for e in range(ci * 4, ci * 4 + 4):
                            nc.tensor.matmul(
                                ps_q[:, :], qsrc[:, e, :], xt[:, e, :],
                                start=(e == 0), stop=(e == NE - 1))
                        if ci == NE // 4 - 1:
                            nc.scalar.activation(
                                QT[:, h, ssl], ps_q[:, :], Id,
                                bias=qb_sb[:, h:h + 1])

                    return [(lambda ci=ci: chunk(ci)) for ci in range(NE // 4)]

                OTb_prev = None
                pending = [None]

                units = [(uq, uh, ukp) for uq in range(NQS)
                         for uh in range(NHL) for ukp in range(NKP)]

                def emit_S(u):
                    uq, uh, ukp = u
                    sps = ps_s.tile([128, 2 * QS], F32, tag="s", name="sps")
                    uqsl = slice(uq * QS, (uq + 1) * QS)
                    for j in (0, 1):
                        kk = 2 * ukp + j
                        nc.tensor.matmul(sps[:, j * QS:(j + 1) * QS],
                                         KT[:, kk * 128:(kk + 1) * 128],
                                         QT[:, uh, uqsl],
                                         start=True, stop=True)
                    return sps

                sps_next = [None]
                unit_idx = [0]
                OTb = None

                def finalize():
                    pqi, ph, ops, acc, pPT = pending[0]
                    pending[0] = None
                    tgt = OTb if pqi == qi_cur[0] else OTb_prev
                    sums = ps_s.tile([1, QS], F32, tag="s", name="sums")
                    last = (ph == NHL - 1)
                    if last:
                        # tail-exposed: last 2 pairs' contribution straight
                        # from PT so sums doesn't wait on their DVE adds
                        nc.tensor.matmul(sums[:, :], ones_col[:, :],
                                         acc[:, :], start=True, stop=False)
                        for kk in range(NKT - 4, NKT):
                            nc.tensor.matmul(sums[:, :], ones_b[:, :],
                                             pPT[:, kk, :], start=False,
                                             stop=(kk == NKT - 1))
                    else:
                        nc.tensor.matmul(sums[:, :], ones_col[:, :],
                                         acc[:, :], start=True, stop=True)
                    recip = attn1.tile([1, QS], F32, tag="recip")
                    bc = attn1.tile([128, QS], F32, tag="bc")
                    tgt[ph] = attn.tile([128, QS], BF16, tag=f"OTb{ph}",
                                        name=f"OTb{ph}")
                    # pipeline the normalization chain in column quarters
                    # across DVE/Pool (the last head's chain gates the next
                    # block's o-proj jobs)
                    for ci in range(4):
                        c = slice(ci * QS // 4, (ci + 1) * QS // 4)
                        nc.vector.reciprocal(recip[:, c], sums[:, c])
                        nc.gpsimd.partition_broadcast(bc[:, c], recip[:, c])
                        nc.vector.tensor_mul(tgt[ph][:, c], ops[:, c],
                                             bc[:, c])

                qi_cur = [0]
                for qi in range(NQS):
                    qi_cur[0] = qi
                    qsl = slice(qi * QS, (qi + 1) * QS)
                    if qi == 0:
                        jobs = (deferred_q_jobs(2, 2)
                                + deferred_q_jobs(2, 3)
                                + deferred_q_jobs(3, 2)
                                + deferred_q_jobs(3, 3))
                    else:
                        # o-projection of the previous q block, interleaved
                        jobs = [
                            (lambda sl=sl, ec=ec, prev=OTb_prev, pqi=qi - 1:
                             oproj_tile(pqi, prev, sl, ec))
                            for sl in range(QS // 128) for ec in range(NEC)]
                    ji = 0
                    OTb = [None] * NHL
                    for h in range(NHL):
                        PT = attn.tile([128, NKT, QS], BF16, tag="PT")
                        ops = ps_o.tile([128, QS], F32, tag="o")
                        acc = attn1.tile([128, QS], F32R, tag="pacc")
                        for kp in range(NKP):
                            k0, k1 = 2 * kp, 2 * kp + 1
                            # S-pairs are emitted one unit ahead (below), so
                            # the exp for this pair was already fed
                            ui = unit_idx[0]
                            unit_idx[0] += 1
                            if ui == 0:
                                sps_next[0] = emit_S(units[0])
                            sps = sps_next[0]
                            nc.scalar.activation(
                                PT[:, k0:k0 + 2, :].rearrange("p k q -> p (k q)"),
                                sps[:, :], Exp, scale=EXP_SCALE)
                            if ui + 1 < len(units):
                                sps_next[0] = emit_S(units[ui + 1])
                            if kp == 0 and pending[0] is not None:
                                # normalization of the previous head, deferred
                                # here so its DVE wait hides under this exp
                                finalize()
                            # fill the exp-latency hole between S and PV
                            if ji < len(jobs):
                                jobs[ji]()
                                ji += 1
                            nc.tensor.matmul(ops[:, :], Vn[:, k0, :],
                                             PT[:, k0, :],
                                             start=(kp == 0), stop=False)
                            nc.tensor.matmul(ops[:, :], Vn[:, k1, :],
                                             PT[:, k1, :],
                                             start=False, stop=(kp == NKP - 1))
                            skip_last = (h == NHL - 1 and kp >= NKP - 2)
                            if kp == 0:
                                nc.vector.tensor_add(acc[:, :], PT[:, 0, :],
                                                     PT[:, 1, :])
                            elif not skip_last:
                                nc.vector.tensor_add(acc[:, :],
                                                     acc[:, :].bitcast(F32),
                                                     PT[:, k0, :])
                                nc.vector.tensor_add(acc[:, :],
                                                     acc[:, :].bitcast(F32),
                                                     PT[:, k1, :])
                        pending[0] = (qi, h, ops, acc, PT)
                        if h == NHL - 1:
                            # finalize immediately: the next q block's first
                            # o-proj jobs need this head's OTb
                            finalize()
                    OTb_prev = OTb

                # last q block's o-projection; final pair DMAs per-tile so
                # the drain overlaps the last copies
                for sl in range(QS // 128):
                    for ec in range(NEC):
                        oproj_tile(NQS - 1, OTb_prev, sl, ec,
                                   split_dma=(sl == QS // 128 - 1 and ec >= 6))

    nc.finalize()
    return nc


def make_in_maps(x, q_w, q_b, k_w, k_b, v_w, v_b, o_w):
    x2 = np.asarray(x, np.float32).reshape(S, E)
    xT = np.ascontiguousarray(x2.T).astype(bfloat16)
    q_w = np.asarray(q_w, np.float32)
    k_w = np.asarray(k_w, np.float32)
    v_w = np.asarray(v_w, np.float32)
    o_w = np.asarray(o_w, np.float32)
    in_maps = []
    for c in range(NCORES):
        qsl = slice(c * DQ, (c + 1) * DQ)
        ksl = slice(c * DKV, (c + 1) * DKV)
        qwT = q_w[qsl].T.astype(bfloat16)                  # [E, 512]

        def pack(w):
            # [E, d] -> SBUF layout [128, NE, d]: (p, ne, j) = w[ne*128+p, j]
            d = w.shape[1]
            return np.ascontiguousarray(
                w.reshape(NE, 128, d).transpose(1, 0, 2))

        in_maps.append({
            "xT": xT,
            "qwp01": pack(qwT[:, 0:2 * D]),
            "qwp23": pack(qwT[:, 2 * D:4 * D]),
            "qb": np.ascontiguousarray(np.asarray(q_b, np.float32)[qsl]),
            "kwp": pack(k_w[ksl].T.astype(bfloat16)),
            "kb": np.ascontiguousarray(np.asarray(k_b, np.float32)[ksl]),
            "vwp": pack(v_w[ksl].T.astype(bfloat16)),
            "vb": np.ascontiguousarray(np.asarray(v_b, np.float32)[ksl]),
            "owT": np.ascontiguousarray(o_w[:, qsl].T).astype(bfloat16),
        })
    return in_maps


def kernel(x, q_w, q_b, k_w, k_b, v_w, v_b, o_w, o_b):
    global _CACHED_NC
    in_maps = make_in_maps(x, q_w, q_b, k_w, k_b, v_w, v_b, o_w)
    if _CACHED_NC is None:
        _CACHED_NC = build_bass()
    res = run_bass_kernel_spmd(_CACHED_NC, in_maps, list(range(NCORES)))
    out = np.zeros((S, E), np.float64)
    for i in range(NCORES):
        out += res.results[i]["out"].astype(np.float64)
    out += np.asarray(o_b, np.float64)
    return out.astype(np.float32).reshape(1, S, E)


# revision 98
# speedup vs baseline: 1.0085x; 1.0012x over previous
"""GQA (softermax) Trainium2 kernel, tensor-parallel over kv-head groups.

Problem: x[1,2048,4096], 32 q-heads / 8 kv-heads, head_dim=128, base-2
softmax (softermax), fp32 io. Each of the 8 cores owns one kv-head group
(4 q-heads, 512 q dims, 128 kv dims) and computes a full partial
o-projection [2048,4096] (fp16 staging); the host sums the 8 partials and
adds o_b.

All matmul operands are pre-transposed on the host and cast to bf16
(fp32 PSUM accumulation), so the device does no layout work except 16
V transposes. Per core:
  proj:  KT/VT/QT[d,s] = (wT chunk)^T @ xT chunk, bias via ACT; weights
         arrive host-packed in SBUF layout (>=1KB descriptor runs; sub-
         512B runs cost 2x DMA bandwidth) interleaved with x pieces in
         consumption order; the first seq block runs K/V/Q as 6
         concurrent PSUM accumulation streams so it consumes the mixed
         DMA stream at arrival rate; Q proj for heads 2-3 x seq-blocks
         2-3 is deferred into the first attention block's pipeline
         holes.
  attn:  S^T[k,q] = KT_chunk^T @ QT (bf16), with every S-pair emitted
         one kt-pair ahead of its exp so PV's wait on the exp is always
         covered by useful PE work; PT = exp2 via ACT over 1024-wide
         pairs -> bf16; O^T[d,q] = sum_kt Vn[kt]^T @ PT[kt];
         denominator: running DVE adds (f32r) + ones-matmul across
         partitions + DVE reciprocal + Pool partition_broadcast + DVE
         mul -> per-head OTb bf16; heads 0-2 defer this normalization
         past the next head's first S-pair, and the last head of each
         q block takes its last two kt-pairs straight from PT via bf16
         ones-matmuls so the chain that gates the next block's o-proj
         jobs doesn't wait on the final DVE adds.
  oproj: po[s,e] psum = sum_h OTb_h^T @ owT_h; one o-proj tile is
         emitted inside every attention kt-pair of the NEXT q block
         (software pipelining that fills the exp-latency hole between
         the S and PV matmuls); PSUM->SBUF copies alternate ACT/DVE into
         fp16 [128,1024] staging tiles, DMA'd 2 tiles per transfer.
"""

import math

import numpy as np
from ml_dtypes import bfloat16

import concourse.bass as bass
from concourse import bacc
import concourse.mybir as mybir
import concourse.tile as tile
from concourse.bass_utils import run_bass_kernel_spmd
from concourse.masks import make_identity

E = 4096          # embed dim
S = 2048          # sequence
D = 128           # head dim
NHL = 4           # q heads per core
DQ = NHL * D      # 512 q dims per core
DKV = 128         # kv dims per core (1 kv head)
NCORES = 8

SB = 512          # seq block for projection pass
NSB = S // SB
QS = 512          # q block in attention
NQS = S // QS
NKT = S // 128    # 16 k chunks
NKP = NKT // 2    # 8 k chunk pairs
NE = E // 128     # 32 embed chunks
EC = 512          # oproj out-column block
NEC = E // EC

F32 = mybir.dt.float32
F16 = mybir.dt.float16
BF16 = mybir.dt.bfloat16
F32R = mybir.dt.float32r
EXP_SCALE = math.log(2.0) / math.sqrt(D)

_CACHED_NC = None


def r(ap):
    return ap.bitcast(F32R)


def build_bass():
    nc = bacc.Bacc(None)

    xT_d = nc.declare_dram_parameter("xT", [E, S], BF16, isOutput=False)
    # weights arrive pre-packed in SBUF layout so every DMA descriptor is a
    # >=1KB contiguous run (sub-512B runs transfer at half bandwidth)
    qwp01_d = nc.declare_dram_parameter("qwp01", [128, NE, 2 * D], BF16,
                                        isOutput=False)
    qwp23_d = nc.declare_dram_parameter("qwp23", [128, NE, 2 * D], BF16,
                                        isOutput=False)
    kwp_d = nc.declare_dram_parameter("kwp", [128, NE, DKV], BF16,
                                      isOutput=False)
    vwp_d = nc.declare_dram_parameter("vwp", [128, NE, DKV], BF16,
                                      isOutput=False)
    owT_d = nc.declare_dram_parameter("owT", [DQ, E], BF16, isOutput=False)
    qb_d = nc.declare_dram_parameter("qb", [DQ], F32, isOutput=False)
    kb_d = nc.declare_dram_parameter("kb", [DKV], F32, isOutput=False)
    vb_d = nc.declare_dram_parameter("vb", [DKV], F32, isOutput=False)
    out_d = nc.declare_dram_parameter("out", [S, E], F16, isOutput=True)

    Id = mybir.ActivationFunctionType.Identity
    Exp = mybir.ActivationFunctionType.Exp

    with tile.TileContext(nc) as tc:
        with (
            tc.tile_pool(name="consts", bufs=1) as consts,
            tc.tile_pool(name="weights", bufs=1) as wpool,
            tc.tile_pool(name="persist", bufs=1) as persist,
        ):
            # ---------------- constants ----------------
            identw = consts.tile([128, 128], BF16)
            make_identity(nc, identw[:, :])
            ones_f = consts.tile([128, 1], F32)
            nc.gpsimd.memset(ones_f[:, :], 1.0)
            ones_col = consts.tile([128, 1], F32R)
            nc.vector.tensor_copy(ones_col[:, :], ones_f[:, :])
            ones_b = consts.tile([128, 1], BF16)
            nc.gpsimd.memset(ones_b[:, :], 1.0)

            qb_sb = consts.tile([128, NHL], F32)
            kb_sb = consts.tile([128, 1], F32)
            vb_sb = consts.tile([128, 1], F32)

            # ---------------- persistent activations ----------------
            KT = persist.tile([128, S], BF16)            # K^T [d, seq]
            Vn = persist.tile([128, NKT, 128], BF16)     # V natural [seq, d]
            QT = persist.tile([128, NHL, S], BF16)       # Q^T per head [d, seq]

            # weight tiles (DMAs interleaved with x blocks, see below)
            kwT = wpool.tile([128, NE, DKV], BF16)
            vwT = wpool.tile([128, NE, DKV], BF16)
            qwT23 = wpool.tile([128, NE, 2 * D], BF16)
            owT = wpool.tile([128, NHL, E], BF16)

            # ---------------- projections ----------------
            with tc.tile_pool(name="xz", bufs=2) as xz:
              with (
                tc.tile_pool(name="wq012", bufs=1) as wq012,
                tc.tile_pool(name="ps_acc", bufs=6, space="PSUM") as ps_acc,
                tc.tile_pool(name="ps_tr", bufs=2, space="PSUM") as ps_tr,
              ):
                qwT = wq012.tile([128, NE, 2 * D], BF16)
                VT = wq012.tile([128, S], BF16)
                # prefetch order: kw/x0 interleaved by quarter, vw, qw per
                # head, x1..x3, owT last — the first K matmul waits on ~0.5 MB
                # and nothing bulky delays the x blocks it gates.
                xts = {}

                def xt_dma(sb, interleave=(), pieces=4):
                    xt = xz.tile([128, NE, SB], BF16, tag="xt", name=f"xt{sb}")
                    ssl = slice(sb * SB, (sb + 1) * SB)
                    ne_p = NE // pieces

                    def wq(dst, wsrc, q):
                        nc.sync.dma_start(
                            dst[:, q * ne_p:(q + 1) * ne_p, :],
                            wsrc[:, q * ne_p:(q + 1) * ne_p, :])

                    for q in range(pieces):
                        # first weight, then x (so the first matmuls can
                        # start), then the remaining weights of this piece
                        if interleave:
                            wq(*interleave[0], q)
                        nc.sync.dma_start(
                            xt[:, q * ne_p:(q + 1) * ne_p, :],
                            xT_d[q * ne_p * 128:(q + 1) * ne_p * 128,
                                 ssl].rearrange("(ne p) s -> p ne s", p=128))
                        for dst, wsrc in interleave[1:]:
                            wq(dst, wsrc, q)
                    xts[sb] = xt

                wq_list = [(kwT, kwp_d), (qwT, qwp01_d),
                           (qwT23, qwp23_d), (vwT, vwp_d)]
                xt_dma(0, interleave=wq_list, pieces=8)
                nc.sync.dma_start(kb_sb[:, :],
                                  kb_d[:].rearrange("(p o) -> p o", o=1))
                nc.sync.dma_start(vb_sb[:, :],
                                  vb_d[:].rearrange("(p o) -> p o", o=1))
                nc.sync.dma_start(qb_sb[:, :],
                                  qb_d[:].rearrange("(t p) -> p t", p=128))
                xt_dma(1)

                for sb in range(NSB):
                    if sb >= 2:
                        xt_dma(sb)
                    if sb == NSB - 1:
                        nc.sync.dma_start(
                            owT[:, :, :],
                            owT_d[:, :].rearrange("(h p) e -> p h e", p=128))
                    xt = xts[sb]
                    ssl = slice(sb * SB, (sb + 1) * SB)
                    qsrcs = [(qwT[:, :, h * 128:(h + 1) * 128] if h < 2
                              else qwT23[:, :, (h - 2) * 128:(h - 1) * 128])
                             for h in range(NHL)]

                    if sb == 0:
                        # 6 concurrent accumulation streams so sb0 consumes
                        # each DMA'd e-chunk eighth for K, V and all Q heads
                        # at once (pure DMA-rate-bound, no ordering stalls)
                        streams = [kwT] + qsrcs + [vwT]
                        pss = [ps_acc.tile([128, SB], F32, tag="acc",
                                           name=f"ps6_{j}")
                               for j in range(6)]
                        for q8 in range(8):
                            for j, w in enumerate(streams):
                                for e in range(q8 * 4, q8 * 4 + 4):
                                    nc.tensor.matmul(
                                        pss[j][:, :], w[:, e, :], xt[:, e, :],
                                        start=(e == 0), stop=(e == NE - 1))
                        nc.scalar.activation(KT[:, ssl], pss[0][:, :], Id,
                                             bias=kb_sb[:, 0:1])
                        for h in range(NHL):
                            nc.scalar.activation(QT[:, h, ssl],
                                                 pss[1 + h][:, :], Id,
                                                 bias=qb_sb[:, h:h + 1])
                        nc.scalar.activation(VT[:, ssl], pss[5][:, :], Id,
                                             bias=vb_sb[:, 0:1])
                    else:
                        ps_k = ps_acc.tile([128, SB], F32, tag="acc")
                        for e in range(NE):
                            nc.tensor.matmul(ps_k[:, :], kwT[:, e, :],
                                             xt[:, e, :],
                                             start=(e == 0), stop=(e == NE - 1))
                        nc.scalar.activation(KT[:, ssl], ps_k[:, :], Id,
                                             bias=kb_sb[:, 0:1])

                        ps_v = ps_acc.tile([128, SB], F32, tag="acc")
                        for e in range(NE):
                            nc.tensor.matmul(ps_v[:, :], vwT[:, e, :],
                                             xt[:, e, :],
                                             start=(e == 0), stop=(e == NE - 1))
                        nc.scalar.activation(VT[:, ssl], ps_v[:, :], Id,
                                             bias=vb_sb[:, 0:1])

                        for h in range(NHL):
                            if h >= 2 and sb >= 2:
                                continue   # deferred into qi0's slots
                            ps_q = ps_acc.tile([128, SB], F32, tag="acc")
                            for e in range(NE):
                                nc.tensor.matmul(
                                    ps_q[:, :], qsrcs[h][:, e, :], xt[:, e, :],
                                    start=(e == 0), stop=(e == NE - 1))
                            nc.scalar.activation(QT[:, h, ssl], ps_q[:, :], Id,
                                                 bias=qb_sb[:, h:h + 1])

                    for i in range(SB // 128):
                        t = sb * (SB // 128) + i
                        tp = ps_tr.tile([128, 128], BF16, tag="tr")
                        nc.tensor.transpose(
                            tp[:, :], VT[:, t * 128:(t + 1) * 128], identw[:, :])
                        nc.vector.tensor_copy(Vn[:, t, :], tp[:, :])

              # ---------------- attention + pipelined o-projection --------
              with (
                  tc.tile_pool(name="attn", bufs=2) as attn,
                  tc.tile_pool(name="attn1", bufs=1) as attn1,
                  tc.tile_pool(name="obp", bufs=3) as obp,
                  tc.tile_pool(name="ps_s", bufs=2, space="PSUM") as ps_s,
                  tc.tile_pool(name="ps_o", bufs=2, space="PSUM") as ps_o,
                  tc.tile_pool(name="ps_po", bufs=2, space="PSUM") as ps_po,
              ):
                ncopy = [0]
                ob_cur = [None]

                def oproj_tile(qi, OTb_q, sl, ec, split_dma=False):
                    # 2 po tiles land in one [128, 2*EC] SBUF tile -> 1 DMA
                    st0 = qi * QS + sl * 128
                    po = ps_po.tile([128, EC], F32, tag="po")
                    for dh in range(NHL):
                        nc.tensor.matmul(
                            po[:, :],
                            OTb_q[dh][:, sl * 128:(sl + 1) * 128],
                            owT[:, dh, ec * EC:(ec + 1) * EC],
                            start=(dh == 0), stop=(dh == NHL - 1))
                    if ec % 2 == 0:
                        ob_cur[0] = obp.tile([128, 2 * EC], F16, tag="ob",
                                             name="ob")
                    ob = ob_cur[0]
                    osl = slice((ec % 2) * EC, (ec % 2 + 1) * EC)
                    i = ncopy[0]
                    ncopy[0] += 1
                    # all copies on ACT: on DVE they would queue ahead of the
                    # adds chain that anchors each head's normalization
                    nc.scalar.copy(ob[:, osl], po[:, :])
                    if split_dma:
                        nc.sync.dma_start(
                            out_d[st0:st0 + 128, ec * EC:(ec + 1) * EC],
                            ob[:, osl])
                    elif ec % 2 == 1:
                        nc.sync.dma_start(
                            out_d[st0:st0 + 128,
                                  (ec - 1) * EC:(ec + 1) * EC], ob[:, :])

                def deferred_q_jobs(sb, h):
                    # Q proj deferred from the projection phase, run inside
                    # qi0's fill slots; borrows the idle po PSUM ring and
                    # the still-resident xt tiles.
                    xt = xts[sb]
                    ssl = slice(sb * SB, (sb + 1) * SB)
                    qsrc = qwT23[:, :, (h - 2) * 128:(h - 1) * 128]
                    st = {}

                    def chunk(ci):
                        if ci == 0:
                            st["ps"] = ps_po.tile([128, SB], F32, tag="po",
                                                  name="ps_qd")
                        ps_q = st["ps"]
                        for e in range(ci * 4, ci * 4 + 4):
                            nc.tensor.matmul(
                                ps_q[:, :], qsrc[:, e, :], xt[:, e, :],
                                start=(e == 0), stop=(e == NE - 1))
                        if ci == NE // 4 - 1:
                            nc.scalar.activation(
                                QT[:, h, ssl], ps_q[:, :], Id,
                                bias=qb_sb[:, h:h + 1])

                    return [(lambda ci=ci: chunk(ci)) for ci in range(NE // 4)]

                OTb_prev = None
                pending = [None]

                units = [(uq, uh, ukp) for uq in range(NQS)
                         for uh in range(NHL) for ukp in range(NKP)]

                def emit_S(u):
                    uq, uh, ukp = u
                    sps = ps_s.tile([128, 2 * QS], F32, tag="s", name="sps")
                    uqsl = slice(uq * QS, (uq + 1) * QS)
                    for j in (0, 1):
                        kk = 2 * ukp + j
                        nc.tensor.matmul(sps[:, j * QS:(j + 1) * QS],
                                         KT[:, kk * 128:(kk + 1) * 128],
                                         QT[:, uh, uqsl],
                                         start=True, stop=True)
                    return sps

                sps_next = [None]
                unit_idx = [0]
                OTb = None

                def finalize():
                    pqi, ph, ops, acc, pPT = pending[0]
                    pending[0] = None
                    tgt = OTb if pqi == qi_cur[0] else OTb_prev
                    sums = ps_s.tile([1, QS], F32, tag="s", name="sums")
                    last = (ph == NHL - 1)
                    if last:
                        # tail-exposed: last 2 pairs' contribution straight
                        # from PT so sums doesn't wait on their DVE adds
                        nc.tensor.matmul(sums[:, :], ones_col[:, :],
                                         acc[:, :], start=True, stop=False)
                        for kk in range(NKT - 4, NKT):
                            nc.tensor.matmul(sums[:, :], ones_b[:, :],
                                             pPT[:, kk, :], start=False,
                                             stop=(kk == NKT - 1))
                    else:
                        nc.tensor.matmul(sums[:, :], ones_col[:, :],
                                         acc[:, :], start=True, stop=True)
                    recip = attn1.tile([1, QS], F32, tag="recip")
                    bc = attn1.tile([128, QS], F32, tag="bc")
                    tgt[ph] = attn.tile([128, QS], BF16, tag=f"OTb{ph}",
                                        name=f"OTb{ph}")
                    # pipeline the normalization chain in column quarters
                    # across DVE/Pool (the last head's chain gates the next
                    # block's o-proj jobs)
                    for ci in range(4):
                        c = slice(ci * QS // 4, (ci + 1) * QS // 4)
                        nc.vector.reciprocal(recip[:, c], sums[:, c])
                        nc.gpsimd.partition_broadcast(bc[:, c], recip[:, c])
                        nc.vector.tensor_mul(tgt[ph][:, c], ops[:, c],
                                             bc[:, c])

                qi_cur = [0]
                for qi in range(NQS):
                    qi_cur[0] = qi
                    qsl = slice(qi * QS, (qi + 1) * QS)
                    if qi == 0:
                        jobs = (deferred_q_jobs(2, 2)
                                + deferred_q_jobs(2, 3)
                                + deferred_q_jobs(3, 2)
                                + deferred_q_jobs(3, 3))
                    else:
                        # o-projection of the previous q block, interleaved
                        jobs = [
                            (lambda sl=sl, ec=ec, prev=OTb_prev, pqi=qi - 1:
                             oproj_tile(pqi, prev, sl, ec))
                            for sl in range(QS // 128) for ec in range(NEC)]
                    ji = 0
                    OTb = [None] * NHL
                    for h in range(NHL):
                        PT = attn.tile([128, NKT, QS], BF16, tag="PT")
                        ops = ps_o.tile([128, QS], F32, tag="o")
                        acc = attn1.tile([128, QS], F32R, tag="pacc", bufs=2)
                        for kp in range(NKP):
                            k0, k1 = 2 * kp, 2 * kp + 1
                            # S-pairs are emitted one unit ahead (below), so
                            # the exp for this pair was already fed
                            ui = unit_idx[0]
                            unit_idx[0] += 1
                            if ui == 0:
                                sps_next[0] = emit_S(units[0])
                            sps = sps_next[0]
                            nc.scalar.activation(
                                PT[:, k0:k0 + 2, :].rearrange("p k q -> p (k q)"),
                                sps[:, :], Exp, scale=EXP_SCALE)
                            if ui + 1 < len(units):
                                sps_next[0] = emit_S(units[ui + 1])
                            if kp == 0 and pending[0] is not None:
                                # normalization of the previous head, deferred
                                # here so its DVE wait hides under this exp
                                finalize()
                            # fill the exp-latency hole between S and PV
                            if ji < len(jobs):
                                jobs[ji]()
                                ji += 1
                            nc.tensor.matmul(ops[:, :], Vn[:, k0, :],
                                             PT[:, k0, :],
                                             start=(kp == 0), stop=False)
                            nc.tensor.matmul(ops[:, :], Vn[:, k1, :],
                                             PT[:, k1, :],
                                             start=False, stop=(kp == NKP - 1))
                            skip_last = (h == NHL - 1 and kp >= NKP - 2)
                            if kp == 0:
                                nc.vector.tensor_add(acc[:, :], PT[:, 0, :],
                                                     PT[:, 1, :])
                            elif not skip_last:
                                nc.vector.tensor_add(acc[:, :],
                                                     acc[:, :].bitcast(F32),
                                                     PT[:, k0, :])
                                nc.vector.tensor_add(acc[:, :],
                                                     acc[:, :].bitcast(F32),
                                                     PT[:, k1, :])
                        pending[0] = (qi, h, ops, acc, PT)
                        if h == NHL - 1:
                            # finalize immediately: the next q block's first
                            # o-proj jobs need this head's OTb
                            finalize()
                    OTb_prev = OTb

                # last q block's o-projection; final pair DMAs per-tile so
                # the drain overlaps the last copies
                for sl in range(QS // 128):
                    for ec in range(NEC):
                        oproj_tile(NQS - 1, OTb_prev, sl, ec,
                                   split_dma=(sl == QS // 128 - 1 and ec >= 6))

    nc.finalize()
    return nc


def make_in_maps(x, q_w, q_b, k_w, k_b, v_w, v_b, o_w):
    x2 = np.asarray(x, np.float32).reshape(S, E)
    xT = np.ascontiguousarray(x2.T).astype(bfloat16)
    q_w = np.asarray(q_w, np.float32)
    k_w = np.asarray(k_w, np.float32)
    v_w = np.asarray(v_w, np.float32)
    o_w = np.asarray(o_w, np.float32)
    in_maps = []
    for c in range(NCORES):
        qsl = slice(c * DQ, (c + 1) * DQ)
        ksl = slice(c * DKV, (c + 1) * DKV)
        qwT = q_w[qsl].T.astype(bfloat16)                  # [E, 512]

        def pack(w):
            # [E, d] -> SBUF layout [128, NE, d]: (p, ne, j) = w[ne*128+p, j]
            d = w.shape[1]
            return np.ascontiguousarray(
                w.reshape(NE, 128, d).transpose(1, 0, 2))

        in_maps.append({
            "xT": xT,
            "qwp01": pack(qwT[:, 0:2 * D]),
            "qwp23": pack(qwT[:, 2 * D:4 * D]),
            "qb": np.ascontiguousarray(np.asarray(q_b, np.float32)[qsl]),
            "kwp": pack(k_w[ksl].T.astype(bfloat16)),
            "kb": np.ascontiguousarray(np.asarray(k_b, np.float32)[ksl]),
            "vwp": pack(v_w[ksl].T.astype(bfloat16)),
            "vb": np.ascontiguousarray(np.asarray(v_b, np.float32)[ksl]),
            "owT": np.ascontiguousarray(o_w[:, qsl].T).astype(bfloat16),
        })
    return in_maps


def kernel(x, q_w, q_b, k_w, k_b, v_w, v_b, o_w, o_b):
    global _CACHED_NC
    in_maps = make_in_maps(x, q_w, q_b, k_w, k_b, v_w, v_b, o_w)
    if _CACHED_NC is None:
        _CACHED_NC = build_bass()
    res = run_bass_kernel_spmd(_CACHED_NC, in_maps, list(range(NCORES)))
    out = np.zeros((S, E), np.float64)
    for i in range(NCORES):
        out += res.results[i]["out"].astype(np.float64)
    out += np.asarray(o_b, np.float64)
    return out.astype(np.float32).reshape(1, S, E)
